# revision 74
# baseline (speedup 1.0000x reference)
"""GSAT graph-attention kernel for 8 Trainium2 NeuronCores.

Math (per batch b):
  h = x @ W                                     [N, 512]
  ss[i] = h[i] . w_src / H ; sd[j] = h[j] . w_dst / H
  t[i,j] = (ss[i] + sd[j]) * adj[i,j] + gumbel(noise[b,i,j])
  A1 = softmax_j(t) ; A2 = softmax_j(A1)
  out[b,n] = sum_i A2[i,n] * (h[i] @ W_out)

Two block flavors, mixed to balance the ACT and DVE engines (the two
softmax Exp passes are ACT-only; DVE customs run at 1 elem/cycle):

DVE flavor (10 blocks) -- with g = gumbel(u) = -log(v), v = -log(u),
exp(t) = exp(m)*exp(g) = exp(m)/v, and since adj is 0/1:
exp(m) = 1 + adj*(E_i*F_j - 1) with E=exp(ss), F=exp(sd):
  L  = Ln(1 - vb)           (ACT; vb = bf16(1-u) streamed, affine fold)
  R  = recip_approx(L)      (DVE custom; R = 1/L < 0; sign cancels)
  w1 = (E_i*F - 1)*adj      (DVE fused affine_mul_reduce)
  e1 = (w1+1)*R, rs1=sum    (DVE affine_mul_reduce, fused accumulate)

ACT flavor (6 blocks) -- classic two-softmax path, no DVE customs:
  L  = Ln(1 - vb), g2 = Ln(-L)      (ACT, back-to-back)
  m  = (sd_j + ss_i)*adj            (DVE affine_mul_reduce)
  e1 = Exp(m - g2), rs1=sum         (DVE 2x-bf16 subtract + ACT Exp)

then for every block:
  e2 = Exp(e1/rs1), rs2=sum (ACT)
  kt = k/rs2                (ACT Copy with per-partition scale)
  outT += kt^T @ e2         (PE, bf16, PSUM-accumulated over blocks)

noise is streamed as vb = bf16(1-u): bf16(u) destroys the u->1 gumbel
tail (winners), but 1-u keeps it exactly (bf16 has full exponent range
for tiny v) -- halves noise DMA; the 1-vb subtract rides the ACT
affine (scale=-1, bias=1) for free.  vb clamped <= 1-2^-8 (bf16-exact;
1-2^-9 would round up to 1.0) so L is never 0/-inf.

A Bacc subclass pins all activations to natural_log_exp_and_others
(holds Ln AND Exp AND Copy): 1 ACT_TABLE_LOAD instead of the greedy
per-function choice that reloaded tables ~2x per block.

Emission order matters (per-engine queues are in-order): block 0/1
DMAs+Ln run ahead of the phase-0 param transfers, and the h/k
PSUM->SBUF copies are emitted after them, so the block-0 chain starts
~10 us in instead of ~36.  GpSimd is left idle on purpose: its 2-input
elementwise floor is 2.6 cyc/elem and it shares SBUF ports with the
DVE, so offloading there slowed the DVE customs by ~16%.

Sharding: 8 cores = (batch b in 0..3) x (row-half rb in 0..1); both
softmaxes are along j so each core computes its 2048 rows completely;
host adds the two row-half partial outputs per batch.
"""

import os
import sys

for _p in ("/opt/trn_rl_repo",):
    if _p not in sys.path and os.path.isdir(_p):
        sys.path.insert(0, _p)

os.environ.setdefault("MYCRO_LOCAL_CACHE", "1")

import numpy as np
import ml_dtypes

B, N, IN_F, H, OUT_F = 4, 4096, 256, 8, 64
D = H * OUT_F          # 512
RB = N // 2            # 2048 rows per core
NBLK = RB // 128       # 16 row blocks per core
EPS = 1e-9
N_CORES = 8
# clamp for vb = bf16(1-u); must be bf16-representable (7 mantissa bits:
# 1-2^-9 would round UP to 1.0 and defeat the clamp)
VMAX = np.float32(1.0 - 2.0 ** -8)

# ACT_FLAVOR blocks use the 4-activation pipeline, which needs no DVE
# customs: each trades ~6.5 us of DVE for ~7.4 us of ACT; 6 of 16
# balances the two engines at ~217 us busy each.
ACT_FLAVOR = {2, 5, 8, 10, 12, 14}
NOISE_TRICK = os.environ.get("NOISE_TRICK", "1") == "1"   # vb=bf16(1-u)

_cache = {}


def _build_module():
    import concourse.bacc as bacc
    import concourse.tile as tile
    from concourse import mybir
    import bass_rust as _bass_rust
    from concourse.hw_specs import get_activation_tables

    f32 = mybir.dt.float32
    bf16 = mybir.dt.bfloat16
    AF = mybir.ActivationFunctionType
    ALU = mybir.AluOpType

    class _Bacc(bacc.Bacc):
        """Pin every activation to the one table set holding Ln AND Exp,
        so the fixpoint pass emits a single ACT_TABLE_LOAD."""

        def insert_act_table_loads(self):
            has_activation = any(
                isinstance(i, mybir.InstActivation)
                for b in self.main_func.blocks
                for i in b.instructions
            )
            if not has_activation:
                return
            real = get_activation_tables(self.m.arch)
            comb = "natural_log_exp_and_others"
            assert comb in real
            tables = [(n, (fns if n == comb else set()))
                      for n, fns in real.items()]
            _bass_rust.insert_act_table_loads(self, tables)

    nc = _Bacc("TRN2", target_bir_lowering=False)

    xT_d = nc.declare_dram_parameter("xT", [IN_F, N], bf16, isOutput=False)
    xTr_d = nc.declare_dram_parameter("xTr", [IN_F, RB], bf16, isOutput=False)
    adj_d = nc.declare_dram_parameter("adj_s", [RB, N], bf16, isOutput=False)
    nz_d = nc.declare_dram_parameter("noise_s", [RB, N],
                                     bf16 if NOISE_TRICK else f32,
                                     isOutput=False)
    W_d = nc.declare_dram_parameter("W", [IN_F, D], bf16, isOutput=False)
    wsd_d = nc.declare_dram_parameter("wsd", [IN_F, 2], bf16, isOutput=False)
    Wo_d = nc.declare_dram_parameter("W_out", [D, OUT_F], bf16, isOutput=False)
    outT_d = nc.declare_dram_parameter("outT", [OUT_F, N], f32, isOutput=True)

    with tile.TileContext(nc) as tc:
        import contextlib
        import concourse.bass as bass_mod

        with contextlib.ExitStack() as ctx:
            pers = ctx.enter_context(tc.tile_pool(name="pers", bufs=1))
            Fb = pers.tile([128, N], bf16)         # exp(sd) broadcast down parts
            sdb = pers.tile([128, N], bf16)        # raw sd broadcast (ACT flavor)
            E_col = pers.tile([128, NBLK], f32)    # exp(ss) per own row
            ss_colr = pers.tile([128, NBLK], f32)  # raw ss (ACT flavor)
            ktil = [pers.tile([128, OUT_F], bf16, tag=f"k{ib}", name=f"k{ib}")
                    for ib in range(NBLK)]
            onesb = pers.tile([128, 1], f32)
            nc.vector.memset(onesb, 1.0 if NOISE_TRICK else EPS)

            # main-loop pools open BEFORE the phase-0 pools so the latter
            # can close (LIFO) before the aggregation pool opens, giving
            # their SBUF back is not possible -- but deeper e1/e2 buffers
            # (3 each) only fit because phase-0's ph/phc/xT2 pools close
            # before the bulk of the main loop runs
            main_ctx = contextlib.ExitStack()
            pnz = main_ctx.enter_context(tc.tile_pool(name="pnz", bufs=2))
            pL = main_ctx.enter_context(tc.tile_pool(name="pL", bufs=2))
            pad = main_ctx.enter_context(tc.tile_pool(name="pad", bufs=2))
            pw = main_ctx.enter_context(tc.tile_pool(name="pw", bufs=2))
            pR = main_ctx.enter_context(tc.tile_pool(name="pR", bufs=2))
            pe1 = main_ctx.enter_context(tc.tile_pool(name="pe1", bufs=3))
            pe2 = main_ctx.enter_context(tc.tile_pool(name="pe2", bufs=3))
            rpool = main_ctx.enter_context(tc.tile_pool(name="smalls", bufs=6))

            # pools that live from phase 0 into the early main loop
            ph_ctx = contextlib.ExitStack()
            ph = ph_ctx.enter_context(tc.tile_pool(name="ph", bufs=1))
            xTr2 = [ph.tile([128, RB], bf16, tag=f"xTr{fc}", name=f"xTr{fc}") for fc in range(2)]
            Wt = [ph.tile([128, D], bf16, tag=f"W{fc}", name=f"Wti{fc}") for fc in range(2)]
            Wot = [ph.tile([128, OUT_F], bf16, tag=f"Wo{dc}", name=f"Wot{dc}") for dc in range(4)]
            wsdt = [ph.tile([128, 2], bf16, tag=f"wsd{fc}", name=f"wsdt{fc}") for fc in range(2)]
            # h is consumed group-wise by the k = h @ W_out matmuls, so it
            # rotates through 512-column chunks instead of living whole
            phc = ph_ctx.enter_context(tc.tile_pool(name="phc", bufs=2))

            def issue_dma(ib):
                nz = pnz.tile([128, N], bf16 if NOISE_TRICK else f32,
                              tag="nz", name=f"nz{ib}")
                nc.sync.dma_start(out=nz, in_=nz_d[ib * 128:(ib + 1) * 128, :])
                ad = pad.tile([128, N], bf16, tag="ad", name=f"ad{ib}")
                nc.sync.dma_start(out=ad, in_=adj_d[ib * 128:(ib + 1) * 128, :])
                return nz, ad



            def stage_front(ib, pre=None):
                # Ln (+recip) only touch nz/onesb, so blocks 0/1 can run
                # this before phase 0 finishes
                nz, ad = pre if pre is not None else issue_dma(ib)
                # L = ln(1 - vb)   (affine folded into the activation)
                Lt = pL.tile([128, N], f32, tag="L", name=f"L{ib}")
                nc.scalar.activation(out=Lt, in_=nz, func=AF.Ln,
                                     bias=onesb,
                                     scale=-1.0 if NOISE_TRICK else 1.0)
                if ib in ACT_FLAVOR:
                    # produce g2 = ln(-L) immediately (ACT back-to-back)
                    # so the DVE's m - g2 subtract never waits on it
                    g2 = pe2.tile([128, N], bf16, tag="e2", name=f"g2{ib}")
                    nc.scalar.activation(out=g2, in_=Lt, func=AF.Ln,
                                         scale=-1.0)
                    return g2, ad
                # R = 1/L (fast approx; R<0, sign cancels in A1)
                Rb = pR.tile([128, N], f32, tag="R", name=f"R{ib}")
                nc.vector.reciprocal_approx_fast(Rb, Lt)
                return Rb, ad

            def stage_a(ib, front=None):
                Rb, ad = front if front is not None else stage_front(ib)
                if ib in ACT_FLAVOR:
                    # classic path, no recip: t = (ss+sd)*adj - ln(-L)
                    # (stt has no 2x uop, so m rides the fused AMR and the
                    # subtract is a plain 2x-bf16 tensor_tensor)
                    g2 = Rb
                    m = pw.tile([128, N], bf16, tag="w1", name=f"m{ib}")
                    macc = rpool.tile([128, 1], f32, tag="wacc")
                    nc.vector.affine_mul_reduce(
                        out=m, accum_out=macc, in0=sdb, in1=ad,
                        scale=1.0, bias=ss_colr[:, ib:ib + 1])
                    e1 = pe1.tile([128, N], bf16, tag="e1", name=f"e1{ib}")
                    nc.vector.tensor_tensor(out=e1, in0=m, in1=g2,
                                            op=ALU.subtract)
                    rs1 = rpool.tile([128, 1], f32, tag="rs1")
                    nc.scalar.activation(out=e1, in_=e1, func=AF.Exp,
                                         accum_out=rs1)
                else:
                    # w1 = (E_i*F_j - 1) * adj
                    w1 = pw.tile([128, N], bf16, tag="w1", name=f"w1{ib}")
                    wacc = rpool.tile([128, 1], f32, tag="wacc")
                    nc.vector.affine_mul_reduce(
                        out=w1, accum_out=wacc, in0=Fb, in1=ad,
                        scale=E_col[:, ib:ib + 1], bias=-1.0)
                    # e1 = (w1 + 1) * R ; rs1 = rowsum(e1)
                    e1 = pe1.tile([128, N], bf16, tag="e1", name=f"e1{ib}")
                    rs1 = rpool.tile([128, 1], f32, tag="rs1")
                    nc.vector.affine_mul_reduce(out=e1, accum_out=rs1,
                                                in0=w1, in1=Rb,
                                                scale=1.0, bias=1.0)
                rs1r = rpool.tile([128, 1], f32, tag="rs1r")
                nc.vector.reciprocal(rs1r, rs1)
                return e1, rs1r

            def stage_b(ib, e1, rs1r, aggp):
                # e2 = exp(e1/rs1), rs2 = rowsum(e2)
                e2 = pe2.tile([128, N], bf16, tag="e2", name=f"e2{ib}")
                rs2 = rpool.tile([128, 1], f32, tag="rs2")
                nc.scalar.activation(out=e2, in_=e1, func=AF.Exp,
                                     scale=rs1r, accum_out=rs2)
                rs2r = rpool.tile([128, 1], f32, tag="rs2r")
                nc.vector.reciprocal(rs2r, rs2)
                # k~ = k / rs2   (ACT Copy with per-partition scale)
                kt = rpool.tile([128, OUT_F], bf16, tag="kt")
                nc.scalar.activation(out=kt, in_=ktil[ib], func=AF.Copy,
                                     scale=rs2r)
                # outT += k~^T @ e2 : accumulate in PSUM across blocks
                for ns in range(8):
                    nc.tensor.matmul(aggp[ns], kt,
                                     e2[:, ns * 512:(ns + 1) * 512],
                                     start=(ib == 0), stop=(ib == NBLK - 1))

            # run Ln+recip for blocks 0/1 ahead of phase 0 so the
            # ACT/DVE queues don't idle behind the param DMAs
            fronts = {ib: stage_front(ib) for ib in (0, 1)}

            # ---------------- phase 0a: scores row/col + params ----------
            # param DMAs ride the Activation engine's hardware queue so
            # they run in parallel with the SP queue's noise/adj stream
            for fc in range(2):
                nc.scalar.dma_start(out=xTr2[fc], in_=xTr_d[fc * 128:(fc + 1) * 128, :])
                nc.scalar.dma_start(out=Wt[fc], in_=W_d[fc * 128:(fc + 1) * 128, :])
                nc.scalar.dma_start(out=wsdt[fc], in_=wsd_d[fc * 128:(fc + 1) * 128, :])
            for dc in range(4):
                nc.scalar.dma_start(out=Wot[dc], in_=Wo_d[dc * 128:(dc + 1) * 128, :])

            # sd row [1, N] -> exp -> broadcast down 128 partitions.
            # xT is streamed one 128-row chunk at a time (saves 8 KB of
            # SBUF for the deeper main-loop buffers); the fc accumulation
            # lives in 8 PSUM tiles, all of PSUM being free at this point
            sd_row = pL.tile([1, N], f32, tag="L", name="sd_row")
            with tc.tile_pool(name="p0", bufs=1) as p0, \
                 tc.tile_pool(name="psd", bufs=1, space="PSUM") as psd:
                sps = [psd.tile([1, 512], f32, tag=f"sps{jc}",
                                name=f"sps{jc}") for jc in range(8)]
                for fc in range(2):
                    xT2 = p0.tile([128, N], bf16, tag="xT", name=f"xT{fc}")
                    nc.scalar.dma_start(out=xT2,
                                        in_=xT_d[fc * 128:(fc + 1) * 128, :])
                    for jc in range(8):
                        nc.tensor.matmul(sps[jc], wsdt[fc][:, 1:2],
                                         xT2[:, jc * 512:(jc + 1) * 512],
                                         start=(fc == 0), stop=(fc == 1))
                for jc in range(8):
                    nc.vector.tensor_copy(
                        sd_row[0:1, jc * 512:(jc + 1) * 512], sps[jc])
            F_row = pe1.tile([1, N], bf16, tag="e1", name="F_row")
            nc.scalar.activation(out=F_row, in_=sd_row, func=AF.Exp)
            F_dram = nc.dram_tensor("F_scratch", [1, N], bf16)
            nc.sync.dma_start(out=F_dram[:], in_=F_row)
            F_bcast = bass_mod.AP(tensor=F_dram[:].tensor,
                                  offset=F_dram[:].offset,
                                  ap=[[0, 128]] + list(F_dram[:].ap)[1:])
            nc.gpsimd.dma_start(out=Fb, in_=F_bcast)
            # raw sd broadcast for the ACT-flavor blocks
            sd_bf = pe2.tile([1, N], bf16, tag="e2", name="sd_bf")
            nc.scalar.copy(sd_bf, sd_row)
            sd_dram = nc.dram_tensor("sd_scratch", [1, N], bf16)
            nc.sync.dma_start(out=sd_dram[:], in_=sd_bf)
            sd_bcast = bass_mod.AP(tensor=sd_dram[:].tensor,
                                   offset=sd_dram[:].offset,
                                   ap=[[0, 128]] + list(sd_dram[:].ap)[1:])
            nc.gpsimd.dma_start(out=sdb, in_=sd_bcast)

            ps0_cm = tc.tile_pool(name="ps0", bufs=2, space="PSUM")
            ps0 = ps0_cm.__enter__()

            # ss_col[p, ib] = ss of row ib*128+p, then E = exp(ss)
            sscol_ps = ps0.tile([128, NBLK], f32, tag="sscol")
            for ib in range(NBLK):
                for fc in range(2):
                    nc.tensor.matmul(sscol_ps[:, ib:ib + 1],
                                     xTr2[fc][:, ib * 128:(ib + 1) * 128],
                                     wsdt[fc][:, 0:1],
                                     start=(fc == 0), stop=(fc == 1))
            nc.scalar.activation(out=E_col, in_=sscol_ps, func=AF.Exp)
            nc.scalar.copy(ss_colr, sscol_ps)

            # kick off blocks 0/1 so ACT's in-order queue reaches Ln(0)
            # before the phase-0b copies below
            staged = {ib: stage_a(ib, fronts.pop(ib)) for ib in (0, 1)}

            # ---------------- phase 0b: h and k = h @ W_out ---------------
            # hT[dc][d, i] = h[i, d] for own rows, one 512-col chunk per
            # 4-block group; ktil mms consume the chunk immediately
            for g in range(NBLK // 4):
                hch = [phc.tile([128, 512], bf16, tag=f"hc{dc}",
                                name=f"hc{dc}_{g}") for dc in range(4)]
                for dc in range(4):
                    hps = ps0.tile([128, 512], f32, tag="hps")
                    for fc in range(2):
                        nc.tensor.matmul(
                            hps,
                            Wt[fc][:, dc * 128:(dc + 1) * 128],
                            xTr2[fc][:, g * 512:(g + 1) * 512],
                            start=(fc == 0), stop=(fc == 1))
                    nc.scalar.copy(hch[dc], hps)
                for ib in range(4 * g, 4 * g + 4):
                    kps = ps0.tile([128, OUT_F], f32, tag="kps")
                    for dc in range(4):
                        nc.tensor.matmul(
                            kps,
                            hch[dc][:, (ib % 4) * 128:(ib % 4 + 1) * 128],
                            Wot[dc],
                            start=(dc == 0), stop=(dc == 3))
                    nc.scalar.copy(ktil[ib], kps)
            ps0_cm.__exit__(None, None, None)
            ph_ctx.close()

            # ---------------- main loop ----------------
            with tc.tile_pool(name="agg", bufs=1, space="PSUM") as aggpool:
                aggp = [aggpool.tile([64, 512], f32, tag=f"agg{j}", name=f"agg{j}")
                        for j in range(8)]
                for ib in range(NBLK):
                    if ib >= 2:
                        staged[ib] = stage_a(ib)
                    e1, rs1r = staged.pop(ib)
                    stage_b(ib, e1, rs1r, aggp)

                # ---------------- epilogue ----------------
                outT = pL.tile([OUT_F, N], f32, tag="L", name="outT")
                for ns in range(8):
                    # split the tail copies across two engines
                    eng = nc.vector.tensor_copy if ns % 2 else nc.scalar.copy
                    eng(outT[:, ns * 512:(ns + 1) * 512], aggp[ns])
                nc.sync.dma_start(out=outT_d[:], in_=outT)
            main_ctx.close()

    nc.compile()
    return nc


def _get_module():
    if "nc" not in _cache:
        _cache["nc"] = _build_module()
    return _cache["nc"]


def kernel(x, adj, noise, W, a_src, a_dst, W_out):
    from concourse.bass_utils import run_bass_kernel_spmd

    nc = _get_module()

    x = np.asarray(x, dtype=np.float32)
    adj = np.asarray(adj, dtype=np.float32)
    noise = np.asarray(noise, dtype=np.float32)
    W = np.asarray(W, dtype=np.float32)
    a_src = np.asarray(a_src, dtype=np.float32)
    a_dst = np.asarray(a_dst, dtype=np.float32)
    W_out = np.asarray(W_out, dtype=np.float32)

    # fold the per-head score weights: s = (x @ W) @ a_flat / H == x @ (W @ a_flat / H)
    w_src = (W @ a_src.reshape(-1)) / H
    w_dst = (W @ a_dst.reshape(-1)) / H
    wsd = np.ascontiguousarray(
        np.stack([w_src, w_dst], axis=1)).astype(ml_dtypes.bfloat16)
    adj_bf = adj.astype(ml_dtypes.bfloat16)  # exact for 0/1 values
    # vb = bf16(1-u), clamped below 1 so ln(1-vb) is never -inf
    if NOISE_TRICK:
        vb = np.minimum((1.0 - noise).astype(ml_dtypes.bfloat16),
                        np.asarray(VMAX, dtype=ml_dtypes.bfloat16))
    else:
        vb = noise
    Wc = np.ascontiguousarray(W).astype(ml_dtypes.bfloat16)
    Woc = np.ascontiguousarray(W_out).astype(ml_dtypes.bfloat16)

    in_maps = []
    for core in range(N_CORES):
        b, rb = core // 2, core % 2
        rows = slice(rb * RB, (rb + 1) * RB)
        xTb = np.ascontiguousarray(x[b].T).astype(ml_dtypes.bfloat16)
        in_maps.append({
            "xT": xTb,
            "xTr": np.ascontiguousarray(xTb[:, rows]),
            "adj_s": np.ascontiguousarray(adj_bf[rows, :]),
            "noise_s": np.ascontiguousarray(vb[b, rows, :]),
            "W": Wc,
            "wsd": wsd,
            "W_out": Woc,
        })

    res = run_bass_kernel_spmd(nc, in_maps, list(range(N_CORES)))
    kernel._last_results = res

    out = np.empty((B, N, OUT_F), dtype=np.float32)
    for b in range(B):
        acc = res.results[2 * b]["outT"].astype(np.float32) + \
            res.results[2 * b + 1]["outT"].astype(np.float32)
        out[b] = acc.T
    return out


# revision 75
# speedup vs baseline: 1.0083x; 1.0083x over previous
"""GSAT graph-attention kernel for 8 Trainium2 NeuronCores.

Math (per batch b):
  h = x @ W                                     [N, 512]
  ss[i] = h[i] . w_src / H ; sd[j] = h[j] . w_dst / H
  t[i,j] = (ss[i] + sd[j]) * adj[i,j] + gumbel(noise[b,i,j])
  A1 = softmax_j(t) ; A2 = softmax_j(A1)
  out[b,n] = sum_i A2[i,n] * (h[i] @ W_out)

Two block flavors, mixed to balance the ACT and DVE engines (the two
softmax Exp passes are ACT-only; DVE customs run at 1 elem/cycle):

DVE flavor (10 blocks) -- with g = gumbel(u) = -log(v), v = -log(u),
exp(t) = exp(m)*exp(g) = exp(m)/v, and since adj is 0/1:
exp(m) = 1 + adj*(E_i*F_j - 1) with E=exp(ss), F=exp(sd):
  L  = Ln(1 - vb)           (ACT; vb = bf16(1-u) streamed, affine fold)
  R  = recip_approx(L)      (DVE custom; R = 1/L < 0; sign cancels)
  w1 = (E_i*F - 1)*adj      (DVE fused affine_mul_reduce)
  e1 = (w1+1)*R, rs1=sum    (DVE affine_mul_reduce, fused accumulate)

ACT flavor (6 blocks) -- classic two-softmax path, no DVE customs:
  L  = Ln(1 - vb), g2 = Ln(-L)      (ACT, back-to-back)
  m  = (sd_j + ss_i)*adj            (DVE affine_mul_reduce)
  e1 = Exp(m - g2), rs1=sum         (DVE 2x-bf16 subtract + ACT Exp)

then for every block:
  e2 = Exp(e1/rs1), rs2=sum (ACT)
  kt = k/rs2                (ACT Copy with per-partition scale)
  outT += kt^T @ e2         (PE, bf16, PSUM-accumulated over blocks)

noise is streamed as vb = bf16(1-u): bf16(u) destroys the u->1 gumbel
tail (winners), but 1-u keeps it exactly (bf16 has full exponent range
for tiny v) -- halves noise DMA; the 1-vb subtract rides the ACT
affine (scale=-1, bias=1) for free.  vb clamped <= 1-2^-8 (bf16-exact;
1-2^-9 would round up to 1.0) so L is never 0/-inf.

A Bacc subclass pins all activations to natural_log_exp_and_others
(holds Ln AND Exp AND Copy): 1 ACT_TABLE_LOAD instead of the greedy
per-function choice that reloaded tables ~2x per block.

Emission order matters (per-engine queues are in-order): block 0/1
DMAs+Ln run ahead of the phase-0 param transfers, and the h/k
PSUM->SBUF copies are emitted after them, so the block-0 chain starts
~10 us in instead of ~36.  GpSimd is left idle on purpose: its 2-input
elementwise floor is 2.6 cyc/elem and it shares SBUF ports with the
DVE, so offloading there slowed the DVE customs by ~16%.

Sharding: 8 cores = (batch b in 0..3) x (row-half rb in 0..1); both
softmaxes are along j so each core computes its 2048 rows completely;
host adds the two row-half partial outputs per batch.
"""

import os
import sys

for _p in ("/opt/trn_rl_repo",):
    if _p not in sys.path and os.path.isdir(_p):
        sys.path.insert(0, _p)

os.environ.setdefault("MYCRO_LOCAL_CACHE", "1")

import numpy as np
import ml_dtypes

B, N, IN_F, H, OUT_F = 4, 4096, 256, 8, 64
D = H * OUT_F          # 512
RB = N // 2            # 2048 rows per core
NBLK = RB // 128       # 16 row blocks per core
EPS = 1e-9
N_CORES = 8
# clamp for vb = bf16(1-u); must be bf16-representable (7 mantissa bits:
# 1-2^-9 would round UP to 1.0 and defeat the clamp)
VMAX = np.float32(1.0 - 2.0 ** -8)

# ACT_FLAVOR blocks use the 4-activation pipeline, which needs no DVE
# customs: each trades ~6.5 us of DVE for ~7.4 us of ACT; 6 of 16
# balances the two engines at ~217 us busy each.
ACT_FLAVOR = {2, 5, 8, 10, 12, 14}
NOISE_TRICK = os.environ.get("NOISE_TRICK", "1") == "1"   # vb=bf16(1-u)

_cache = {}


def _build_module():
    import concourse.bacc as bacc
    import concourse.tile as tile
    from concourse import mybir
    import bass_rust as _bass_rust
    from concourse.hw_specs import get_activation_tables

    f32 = mybir.dt.float32
    bf16 = mybir.dt.bfloat16
    AF = mybir.ActivationFunctionType
    ALU = mybir.AluOpType

    class _Bacc(bacc.Bacc):
        """Pin every activation to the one table set holding Ln AND Exp,
        so the fixpoint pass emits a single ACT_TABLE_LOAD."""

        def insert_act_table_loads(self):
            has_activation = any(
                isinstance(i, mybir.InstActivation)
                for b in self.main_func.blocks
                for i in b.instructions
            )
            if not has_activation:
                return
            real = get_activation_tables(self.m.arch)
            comb = "natural_log_exp_and_others"
            assert comb in real
            tables = [(n, (fns if n == comb else set()))
                      for n, fns in real.items()]
            _bass_rust.insert_act_table_loads(self, tables)

    nc = _Bacc("TRN2", target_bir_lowering=False)

    xT_d = nc.declare_dram_parameter("xT", [IN_F, N], bf16, isOutput=False)
    xTr_d = nc.declare_dram_parameter("xTr", [IN_F, RB], bf16, isOutput=False)
    adj_d = nc.declare_dram_parameter("adj_s", [RB, N], bf16, isOutput=False)
    nz_d = nc.declare_dram_parameter("noise_s", [RB, N],
                                     bf16 if NOISE_TRICK else f32,
                                     isOutput=False)
    W_d = nc.declare_dram_parameter("W", [IN_F, D], bf16, isOutput=False)
    wsd_d = nc.declare_dram_parameter("wsd", [IN_F, 2], bf16, isOutput=False)
    Wo_d = nc.declare_dram_parameter("W_out", [D, OUT_F], bf16, isOutput=False)
    outT_d = nc.declare_dram_parameter("outT", [OUT_F, N], f32, isOutput=True)

    with tile.TileContext(nc) as tc:
        import contextlib
        import concourse.bass as bass_mod

        with contextlib.ExitStack() as ctx:
            pers = ctx.enter_context(tc.tile_pool(name="pers", bufs=1))
            Fb = pers.tile([128, N], bf16)         # exp(sd) broadcast down parts
            sdb = pers.tile([128, N], bf16)        # raw sd broadcast (ACT flavor)
            E_col = pers.tile([128, NBLK], f32)    # exp(ss) per own row
            ss_colr = pers.tile([128, NBLK], f32)  # raw ss (ACT flavor)
            ktil = [pers.tile([128, OUT_F], bf16, tag=f"k{ib}", name=f"k{ib}")
                    for ib in range(NBLK)]
            onesb = pers.tile([128, 1], f32)
            nc.vector.memset(onesb, 1.0 if NOISE_TRICK else EPS)

            # main-loop pools open BEFORE the phase-0 pools so the latter
            # can close (LIFO) before the aggregation pool opens, giving
            # their SBUF back is not possible -- but deeper e1/e2 buffers
            # (3 each) only fit because phase-0's ph/phc/xT2 pools close
            # before the bulk of the main loop runs
            main_ctx = contextlib.ExitStack()
            pnz = main_ctx.enter_context(tc.tile_pool(name="pnz", bufs=2))
            pL = main_ctx.enter_context(tc.tile_pool(name="pL", bufs=2))
            pad = main_ctx.enter_context(tc.tile_pool(name="pad", bufs=2))
            pw = main_ctx.enter_context(tc.tile_pool(name="pw", bufs=2))
            pR = main_ctx.enter_context(tc.tile_pool(name="pR", bufs=2))
            pe1 = main_ctx.enter_context(tc.tile_pool(name="pe1", bufs=3))
            pe2 = main_ctx.enter_context(tc.tile_pool(name="pe2", bufs=3))
            rpool = main_ctx.enter_context(tc.tile_pool(name="smalls", bufs=6))

            # pools that live from phase 0 into the early main loop
            ph_ctx = contextlib.ExitStack()
            ph = ph_ctx.enter_context(tc.tile_pool(name="ph", bufs=1))
            xTr2 = [ph.tile([128, RB], bf16, tag=f"xTr{fc}", name=f"xTr{fc}") for fc in range(2)]
            Wt = [ph.tile([128, D], bf16, tag=f"W{fc}", name=f"Wti{fc}") for fc in range(2)]
            Wot = [ph.tile([128, OUT_F], bf16, tag=f"Wo{dc}", name=f"Wot{dc}") for dc in range(4)]
            wsdt = [ph.tile([128, 2], bf16, tag=f"wsd{fc}", name=f"wsdt{fc}") for fc in range(2)]
            # h is consumed group-wise by the k = h @ W_out matmuls, so it
            # rotates through 512-column chunks instead of living whole
            phc = ph_ctx.enter_context(tc.tile_pool(name="phc", bufs=2))

            def issue_dma(ib):
                nz = pnz.tile([128, N], bf16 if NOISE_TRICK else f32,
                              tag="nz", name=f"nz{ib}")
                nc.sync.dma_start(out=nz, in_=nz_d[ib * 128:(ib + 1) * 128, :])
                ad = pad.tile([128, N], bf16, tag="ad", name=f"ad{ib}")
                nc.sync.dma_start(out=ad, in_=adj_d[ib * 128:(ib + 1) * 128, :])
                return nz, ad



            def stage_front(ib, pre=None):
                # Ln (+recip) only touch nz/onesb, so blocks 0/1 can run
                # this before phase 0 finishes
                nz, ad = pre if pre is not None else issue_dma(ib)
                # L = ln(1 - vb)   (affine folded into the activation)
                Lt = pL.tile([128, N], f32, tag="L", name=f"L{ib}")
                nc.scalar.activation(out=Lt, in_=nz, func=AF.Ln,
                                     bias=onesb,
                                     scale=-1.0 if NOISE_TRICK else 1.0)
                if ib in ACT_FLAVOR:
                    # produce g2 = ln(-L) immediately (ACT back-to-back)
                    # so the DVE's m - g2 subtract never waits on it
                    g2 = pe2.tile([128, N], bf16, tag="e2", name=f"g2{ib}")
                    nc.scalar.activation(out=g2, in_=Lt, func=AF.Ln,
                                         scale=-1.0)
                    return g2, ad
                # R = 1/L (fast approx; R<0, sign cancels in A1)
                Rb = pR.tile([128, N], f32, tag="R", name=f"R{ib}")
                nc.vector.reciprocal_approx_fast(Rb, Lt)
                return Rb, ad

            def stage_a(ib, front=None):
                Rb, ad = front if front is not None else stage_front(ib)
                if ib in ACT_FLAVOR:
                    # classic path, no recip: t = (ss+sd)*adj - ln(-L)
                    # (stt has no 2x uop, so m rides the fused AMR and the
                    # subtract is a plain 2x-bf16 tensor_tensor)
                    g2 = Rb
                    m = pw.tile([128, N], bf16, tag="w1", name=f"m{ib}")
                    macc = rpool.tile([128, 1], f32, tag="wacc")
                    nc.vector.affine_mul_reduce(
                        out=m, accum_out=macc, in0=sdb, in1=ad,
                        scale=1.0, bias=ss_colr[:, ib:ib + 1])
                    e1 = pe1.tile([128, N], bf16, tag="e1", name=f"e1{ib}")
                    nc.vector.tensor_tensor(out=e1, in0=m, in1=g2,
                                            op=ALU.subtract)
                    rs1 = rpool.tile([128, 1], f32, tag="rs1")
                    nc.scalar.activation(out=e1, in_=e1, func=AF.Exp,
                                         accum_out=rs1)
                else:
                    # w1 = (E_i*F_j - 1) * adj
                    w1 = pw.tile([128, N], bf16, tag="w1", name=f"w1{ib}")
                    wacc = rpool.tile([128, 1], f32, tag="wacc")
                    nc.vector.affine_mul_reduce(
                        out=w1, accum_out=wacc, in0=Fb, in1=ad,
                        scale=E_col[:, ib:ib + 1], bias=-1.0)
                    # e1 = (w1 + 1) * R ; rs1 = rowsum(e1)
                    e1 = pe1.tile([128, N], bf16, tag="e1", name=f"e1{ib}")
                    rs1 = rpool.tile([128, 1], f32, tag="rs1")
                    nc.vector.affine_mul_reduce(out=e1, accum_out=rs1,
                                                in0=w1, in1=Rb,
                                                scale=1.0, bias=1.0)
                rs1r = rpool.tile([128, 1], f32, tag="rs1r")
                nc.vector.reciprocal(rs1r, rs1)
                return e1, rs1r

            def stage_b(ib, e1, rs1r, aggp):
                # e2 = exp(e1/rs1), rs2 = rowsum(e2)
                e2 = pe2.tile([128, N], bf16, tag="e2", name=f"e2{ib}")
                rs2 = rpool.tile([128, 1], f32, tag="rs2")
                nc.scalar.activation(out=e2, in_=e1, func=AF.Exp,
                                     scale=rs1r, accum_out=rs2)
                rs2r = rpool.tile([128, 1], f32, tag="rs2r")
                nc.vector.reciprocal(rs2r, rs2)
                # k~ = k / rs2   (ACT Copy with per-partition scale)
                kt = rpool.tile([128, OUT_F], bf16, tag="kt")
                nc.scalar.activation(out=kt, in_=ktil[ib], func=AF.Copy,
                                     scale=rs2r)
                # outT += k~^T @ e2 : accumulate in PSUM across blocks
                for ns in range(8):
                    nc.tensor.matmul(aggp[ns], kt,
                                     e2[:, ns * 512:(ns + 1) * 512],
                                     start=(ib == 0), stop=(ib == NBLK - 1))

            # run Ln+recip for blocks 0/1 ahead of phase 0 so the
            # ACT/DVE queues don't idle behind the param DMAs
            fronts = {ib: stage_front(ib) for ib in (0, 1)}

            # ---------------- phase 0a: scores row/col + params ----------
            for fc in range(2):
                nc.sync.dma_start(out=xTr2[fc], in_=xTr_d[fc * 128:(fc + 1) * 128, :])
                nc.sync.dma_start(out=Wt[fc], in_=W_d[fc * 128:(fc + 1) * 128, :])
                nc.sync.dma_start(out=wsdt[fc], in_=wsd_d[fc * 128:(fc + 1) * 128, :])
            for dc in range(4):
                nc.sync.dma_start(out=Wot[dc], in_=Wo_d[dc * 128:(dc + 1) * 128, :])

            # sd row [1, N] -> exp -> broadcast down 128 partitions.
            # xT is streamed one 128-row chunk at a time (saves 8 KB of
            # SBUF for the deeper main-loop buffers); the fc accumulation
            # lives in 8 PSUM tiles, all of PSUM being free at this point
            sd_row = pL.tile([1, N], f32, tag="L", name="sd_row")
            with tc.tile_pool(name="p0", bufs=1) as p0, \
                 tc.tile_pool(name="psd", bufs=1, space="PSUM") as psd:
                sps = [psd.tile([1, 512], f32, tag=f"sps{jc}",
                                name=f"sps{jc}") for jc in range(8)]
                for fc in range(2):
                    xT2 = p0.tile([128, N], bf16, tag="xT", name=f"xT{fc}")
                    nc.sync.dma_start(out=xT2,
                                      in_=xT_d[fc * 128:(fc + 1) * 128, :])
                    for jc in range(8):
                        nc.tensor.matmul(sps[jc], wsdt[fc][:, 1:2],
                                         xT2[:, jc * 512:(jc + 1) * 512],
                                         start=(fc == 0), stop=(fc == 1))
                for jc in range(8):
                    nc.vector.tensor_copy(
                        sd_row[0:1, jc * 512:(jc + 1) * 512], sps[jc])
            F_row = pe1.tile([1, N], bf16, tag="e1", name="F_row")
            nc.scalar.activation(out=F_row, in_=sd_row, func=AF.Exp)
            F_dram = nc.dram_tensor("F_scratch", [1, N], bf16)
            nc.sync.dma_start(out=F_dram[:], in_=F_row)
            F_bcast = bass_mod.AP(tensor=F_dram[:].tensor,
                                  offset=F_dram[:].offset,
                                  ap=[[0, 128]] + list(F_dram[:].ap)[1:])
            nc.gpsimd.dma_start(out=Fb, in_=F_bcast)
            # raw sd broadcast for the ACT-flavor blocks
            sd_bf = pe2.tile([1, N], bf16, tag="e2", name="sd_bf")
            nc.scalar.copy(sd_bf, sd_row)
            sd_dram = nc.dram_tensor("sd_scratch", [1, N], bf16)
            nc.sync.dma_start(out=sd_dram[:], in_=sd_bf)
            sd_bcast = bass_mod.AP(tensor=sd_dram[:].tensor,
                                   offset=sd_dram[:].offset,
                                   ap=[[0, 128]] + list(sd_dram[:].ap)[1:])
            nc.gpsimd.dma_start(out=sdb, in_=sd_bcast)

            ps0_cm = tc.tile_pool(name="ps0", bufs=2, space="PSUM")
            ps0 = ps0_cm.__enter__()

            # ss_col[p, ib] = ss of row ib*128+p, then E = exp(ss)
            sscol_ps = ps0.tile([128, NBLK], f32, tag="sscol")
            for ib in range(NBLK):
                for fc in range(2):
                    nc.tensor.matmul(sscol_ps[:, ib:ib + 1],
                                     xTr2[fc][:, ib * 128:(ib + 1) * 128],
                                     wsdt[fc][:, 0:1],
                                     start=(fc == 0), stop=(fc == 1))
            nc.scalar.activation(out=E_col, in_=sscol_ps, func=AF.Exp)
            nc.scalar.copy(ss_colr, sscol_ps)

            # kick off blocks 0/1 so ACT's in-order queue reaches Ln(0)
            # before the phase-0b copies below
            staged = {ib: stage_a(ib, fronts.pop(ib)) for ib in (0, 1)}

            # ---------------- phase 0b: h and k = h @ W_out ---------------
            # hT[dc][d, i] = h[i, d] for own rows, one 512-col chunk per
            # 4-block group; ktil mms consume the chunk immediately
            for g in range(NBLK // 4):
                hch = [phc.tile([128, 512], bf16, tag=f"hc{dc}",
                                name=f"hc{dc}_{g}") for dc in range(4)]
                for dc in range(4):
                    hps = ps0.tile([128, 512], f32, tag="hps")
                    for fc in range(2):
                        nc.tensor.matmul(
                            hps,
                            Wt[fc][:, dc * 128:(dc + 1) * 128],
                            xTr2[fc][:, g * 512:(g + 1) * 512],
                            start=(fc == 0), stop=(fc == 1))
                    nc.scalar.copy(hch[dc], hps)
                for ib in range(4 * g, 4 * g + 4):
                    kps = ps0.tile([128, OUT_F], f32, tag="kps")
                    for dc in range(4):
                        nc.tensor.matmul(
                            kps,
                            hch[dc][:, (ib % 4) * 128:(ib % 4 + 1) * 128],
                            Wot[dc],
                            start=(dc == 0), stop=(dc == 3))
                    nc.scalar.copy(ktil[ib], kps)
            ps0_cm.__exit__(None, None, None)
            ph_ctx.close()

            # ---------------- main loop ----------------
            with tc.tile_pool(name="agg", bufs=1, space="PSUM") as aggpool:
                aggp = [aggpool.tile([64, 512], f32, tag=f"agg{j}", name=f"agg{j}")
                        for j in range(8)]
                for ib in range(NBLK):
                    if ib >= 2:
                        staged[ib] = stage_a(ib)
                    e1, rs1r = staged.pop(ib)
                    stage_b(ib, e1, rs1r, aggp)

                # ---------------- epilogue ----------------
                outT = pL.tile([OUT_F, N], f32, tag="L", name="outT")
                for ns in range(8):
                    # split the tail copies across two engines
                    eng = nc.vector.tensor_copy if ns % 2 else nc.scalar.copy
                    eng(outT[:, ns * 512:(ns + 1) * 512], aggp[ns])
                nc.sync.dma_start(out=outT_d[:], in_=outT)
            main_ctx.close()

    nc.compile()
    return nc


def _get_module():
    if "nc" not in _cache:
        _cache["nc"] = _build_module()
    return _cache["nc"]


def kernel(x, adj, noise, W, a_src, a_dst, W_out):
    from concourse.bass_utils import run_bass_kernel_spmd

    nc = _get_module()

    x = np.asarray(x, dtype=np.float32)
    adj = np.asarray(adj, dtype=np.float32)
    noise = np.asarray(noise, dtype=np.float32)
    W = np.asarray(W, dtype=np.float32)
    a_src = np.asarray(a_src, dtype=np.float32)
    a_dst = np.asarray(a_dst, dtype=np.float32)
    W_out = np.asarray(W_out, dtype=np.float32)

    # fold the per-head score weights: s = (x @ W) @ a_flat / H == x @ (W @ a_flat / H)
    w_src = (W @ a_src.reshape(-1)) / H
    w_dst = (W @ a_dst.reshape(-1)) / H
    wsd = np.ascontiguousarray(
        np.stack([w_src, w_dst], axis=1)).astype(ml_dtypes.bfloat16)
    adj_bf = adj.astype(ml_dtypes.bfloat16)  # exact for 0/1 values
    # vb = bf16(1-u), clamped below 1 so ln(1-vb) is never -inf
    if NOISE_TRICK:
        vb = np.minimum((1.0 - noise).astype(ml_dtypes.bfloat16),
                        np.asarray(VMAX, dtype=ml_dtypes.bfloat16))
    else:
        vb = noise
    Wc = np.ascontiguousarray(W).astype(ml_dtypes.bfloat16)
    Woc = np.ascontiguousarray(W_out).astype(ml_dtypes.bfloat16)

    in_maps = []
    for core in range(N_CORES):
        b, rb = core // 2, core % 2
        rows = slice(rb * RB, (rb + 1) * RB)
        xTb = np.ascontiguousarray(x[b].T).astype(ml_dtypes.bfloat16)
        in_maps.append({
            "xT": xTb,
            "xTr": np.ascontiguousarray(xTb[:, rows]),
            "adj_s": np.ascontiguousarray(adj_bf[rows, :]),
            "noise_s": np.ascontiguousarray(vb[b, rows, :]),
            "W": Wc,
            "wsd": wsd,
            "W_out": Woc,
        })

    res = run_bass_kernel_spmd(nc, in_maps, list(range(N_CORES)))
    kernel._last_results = res

    out = np.empty((B, N, OUT_F), dtype=np.float32)
    for b in range(B):
        acc = res.results[2 * b]["outT"].astype(np.float32) + \
            res.results[2 * b + 1]["outT"].astype(np.float32)
        out[b] = acc.T
    return out
